# revision 4
# baseline (speedup 1.0000x reference)
"""CharRNN Trainium2 kernel.

Reference computation (per batch row b, t = 0..T-1):
    x_t   = W_ih.T[token[b,t]] + b_ih + b_hh          # row gather  [H]
    h_t   = tanh(x_t + W_hh @ h_{t-1})                # recurrence  [H]
    logit = h_t @ W_fc.T + b_fc                       # output head [V]

Shapes: B=64, T=512, H=512, V=4096.  8 NeuronCores, data-parallel over B
(8 rows per core), zero communication.

Per-core device program (one TileContext, fully unrolled):
  - embedding gather via indirect DMA (128 rows/block, 32 blocks), PE
    transpose into H-partition layout, bias folded into the PSUM-evacuate
    DVE add (fp32).
  - recurrence keeps h in transposed layout hT [H on partitions, B free].
    Per step: 4 fp32 identity-matmuls inject x into PSUM, 16 bf16 matmuls
    accumulate W_hh.T chunks, one ACT tanh writes hT back to SBUF (bf16).
  - output head: hsT tranche tiles serve as stationary lhsT (128-row
    blocks), W_fc.T streams as moving operand (bf16), bias added by the
    mandatory PSUM->SBUF DVE evacuation, 2MB contiguous DMA per row block.
"""

import threading

import numpy as np

B, T, H, V = 64, 512, 512, 4096
NCORES = 8
BC = B // NCORES          # 8 batch rows per core
HC = H // 128             # 4 hidden chunks
VC = V // 512             # 8 vocab chunks of 512
TBLK = 16                 # gather block = 16 steps * 8 rows = 128 gathered rows
NBLK = T // TBLK          # 32 gather blocks
TRANCHE = 128             # logits row-block granularity (steps per hsT tile)
NTR = T // TRANCHE        # 4 tranches


def _build_nc():
    import concourse.bacc as bacc
    import concourse.mybir as mybir
    import concourse.tile as tile
    from concourse import bass
    from concourse.masks import make_identity
    from contextlib import ExitStack

    f32 = mybir.dt.float32
    bf16 = mybir.dt.bfloat16
    i32 = mybir.dt.int32

    nc = bacc.Bacc("TRN2", target_bir_lowering=False, debug=False,
                   num_devices=NCORES)

    # ---- external I/O (per-core) ----
    tok_pk = nc.declare_dram_parameter("tok_pk", [128, NBLK], i32, isOutput=False)
    w_ihT = nc.declare_dram_parameter("w_ihT", [V, H], f32, isOutput=False)
    whh_pk = nc.declare_dram_parameter("whh_pk", [128, 16 * 128], f32, isOutput=False)
    wfc_pk = nc.declare_dram_parameter("wfc_pk", [128, HC * V], f32, isOutput=False)
    bih_pk = nc.declare_dram_parameter("bih_pk", [128, HC], f32, isOutput=False)
    bhh_pk = nc.declare_dram_parameter("bhh_pk", [128, HC], f32, isOutput=False)
    bfc_rep = nc.declare_dram_parameter("bfc_rep", [128, V], f32, isOutput=False)
    h0_pk = nc.declare_dram_parameter("h0_pk", [128, HC * BC], f32, isOutput=False)
    logits = nc.declare_dram_parameter("logits", [BC * T, V], f32, isOutput=True)
    h_out = nc.declare_dram_parameter("h_out", [BC, H], f32, isOutput=True)

    with ExitStack() as ctx:
        tc = ctx.enter_context(tile.TileContext(nc))
        const = ctx.enter_context(tc.tile_pool(name="const", bufs=1))
        gpool = ctx.enter_context(tc.tile_pool(name="gather", bufs=3))
        xpool = ctx.enter_context(tc.tile_pool(name="xp", bufs=3))
        opool = ctx.enter_context(tc.tile_pool(name="outst", bufs=2))
        tpool = ctx.enter_context(tc.tile_pool(name="xt_ps", bufs=2, space="PSUM"))
        zpool = ctx.enter_context(tc.tile_pool(name="z_ps", bufs=3, space="PSUM"))
        lpool = ctx.enter_context(tc.tile_pool(name="l_ps", bufs=3, space="PSUM"))

        # ---- prep: weights into SBUF (bf16 via SWDGE cast-DMA) ----
        whh_bf = const.tile([128, 16 * 128], bf16, tag="whh")
        nc.gpsimd.dma_start(out=whh_bf[:], in_=whh_pk[:])
        wfc_bf = const.tile([128, HC * V], bf16, tag="wfc")
        nc.gpsimd.dma_start(out=wfc_bf[:], in_=wfc_pk[:])
        h0_bf = const.tile([128, HC * BC], bf16, tag="h0")
        nc.gpsimd.dma_start(out=h0_bf[:], in_=h0_pk[:])

        bias_a = const.tile([128, HC], f32, tag="bias_a")
        bias_b = const.tile([128, HC], f32, tag="bias_b")
        nc.sync.dma_start(out=bias_a[:], in_=bih_pk[:])
        nc.sync.dma_start(out=bias_b[:], in_=bhh_pk[:])
        bias_pk = const.tile([128, HC], f32, tag="bias")
        nc.vector.tensor_add(out=bias_pk[:], in0=bias_a[:], in1=bias_b[:])

        bfc_sb = const.tile([128, V], f32, tag="bfc")
        nc.sync.dma_start(out=bfc_sb[:], in_=bfc_rep[:])

        tok_sb = const.tile([128, NBLK], i32, tag="tok")
        nc.sync.dma_start(out=tok_sb[:], in_=tok_pk[:])

        ident_f = const.tile([128, 128], f32, tag="id_f")
        make_identity(nc, ident_f[:])
        ident_b = const.tile([128, 128], bf16, tag="id_b")
        nc.vector.tensor_copy(out=ident_b[:], in_=ident_f[:])

        # hsT tranches: [128, HC * BC * TRANCHE] bf16; column = c*1024 + b*128 + (t%128)
        hs = [const.tile([128, HC * BC * TRANCHE], bf16, tag=f"hs{i}", name=f"hs{i}")
              for i in range(NTR)]
        hs_v = [h[:].rearrange("p (c b t) -> p c b t", c=HC, b=BC, t=TRANCHE)
                for h in hs]

        xp_tiles = {}  # blk -> [4 tiles of [128, TBLK*BC] f32]

        def emit_gather(blk):
            xg = gpool.tile([128, H], f32, tag="xg", name="xg")
            nc.gpsimd.indirect_dma_start(
                out=xg[:],
                out_offset=None,
                in_=w_ihT[:],
                in_offset=bass.IndirectOffsetOnAxis(
                    ap=tok_sb[:, blk:blk + 1], axis=0),
            )
            tiles = []
            for c in range(HC):
                ps = tpool.tile([128, 128], f32, tag="xt", name="xt")
                nc.tensor.transpose(out=ps[:], in_=xg[:, c * 128:(c + 1) * 128],
                                    identity=ident_f[:])
                xp = xpool.tile([128, TBLK * BC], f32, tag=f"xp{c}", name=f"xp{c}")
                nc.vector.tensor_add(
                    out=xp[:], in0=ps[:],
                    in1=bias_pk[:, c:c + 1].to_broadcast([128, TBLK * BC]))
                tiles.append(xp)
            xp_tiles[blk] = tiles

        def h_prev(k, t):
            if t == 0:
                return h0_bf[:, k * BC:(k + 1) * BC]
            tr, tt = (t - 1) // TRANCHE, (t - 1) % TRANCHE
            return hs_v[tr][:, k, :, tt]

        def emit_step(t):
            blk, j = t // TBLK, t % TBLK
            z = zpool.tile([128, HC * BC], f32, tag="z", name="z")
            xts = xp_tiles[blk]
            for m in range(HC):
                nc.tensor.matmul(
                    out=z[:, m * BC:(m + 1) * BC], lhsT=ident_f[:],
                    rhs=xts[m][:, j * BC:(j + 1) * BC],
                    start=(m == 0), stop=False)
            for k in range(HC):
                for m in range(HC):
                    nc.tensor.matmul(
                        out=z[:, m * BC:(m + 1) * BC],
                        lhsT=whh_bf[:, (k * HC + m) * 128:(k * HC + m + 1) * 128],
                        rhs=h_prev(k, t),
                        start=False, stop=(k == HC - 1 and m == HC - 1))
            tr, tt = t // TRANCHE, t % TRANCHE
            nc.scalar.activation(
                out=hs_v[tr][:, :, :, tt],
                in_=z[:].rearrange("p (c b) -> p c b", c=HC),
                func=mybir.ActivationFunctionType.Tanh)
            if t == T - 1:
                hl = const.tile([128, HC * BC], f32, tag="hlast", name="hlast")
                nc.scalar.activation(out=hl[:], in_=z[:],
                                     func=mybir.ActivationFunctionType.Tanh)
                for c in range(HC):
                    nc.sync.dma_start(
                        out=h_out[:, c * 128:(c + 1) * 128].rearrange(
                            "b p -> p b"),
                        in_=hl[:, c * BC:(c + 1) * BC])

        def emit_logits_tranche(tr):
            for b in range(BC):
                outst = opool.tile([128, V], f32, tag="outst", name="outst")
                for n in range(VC):
                    ps = lpool.tile([128, 512], f32, tag="lg", name="lg")
                    for k in range(HC):
                        nc.tensor.matmul(
                            out=ps[:],
                            lhsT=hs_v[tr][:, k, b, :],
                            rhs=wfc_bf[:, k * V + n * 512: k * V + (n + 1) * 512],
                            start=(k == 0), stop=(k == HC - 1))
                    nc.vector.tensor_add(
                        out=outst[:, n * 512:(n + 1) * 512], in0=ps[:],
                        in1=bfc_sb[:, n * 512:(n + 1) * 512])
                row0 = b * T + tr * TRANCHE
                nc.sync.dma_start(out=logits[row0:row0 + TRANCHE, :],
                                  in_=outst[:])

        # ---- main schedule ----
        emit_gather(0)
        emit_gather(1)
        for t in range(T):
            if t % TBLK == 0 and t // TBLK + 2 < NBLK:
                emit_gather(t // TBLK + 2)
            emit_step(t)
            if t % TRANCHE == TRANCHE - 1:
                emit_logits_tranche(t // TRANCHE)

    nc.compile()
    return nc


_cache = threading.local()


def _get_runner():
    if getattr(_cache, "runner", None) is None:
        nc = _build_nc()
        _cache.runner = nc
    return _cache.runner


def _pack_inputs(input, hidden, W_ih, W_hh, b_ih, b_hh, W_fc, b_fc):
    """Host-side layout-only packing (slicing / transpose / replication)."""
    tok = np.asarray(input).astype(np.int32)           # [B, T]
    hid = np.ascontiguousarray(np.asarray(hidden, np.float32)[0])  # [B, H]
    w_ihT = np.ascontiguousarray(np.asarray(W_ih, np.float32).T)   # [V, H]
    w_hhT = np.asarray(W_hh, np.float32).T                         # [H, H]
    # whh_pk column block (k*HC+m) = W_hh.T[k*128:(k+1)*128, m*128:(m+1)*128]
    whh_pk = np.concatenate(
        [w_hhT[k * 128:(k + 1) * 128, m * 128:(m + 1) * 128]
         for k in range(HC) for m in range(HC)], axis=1)
    whh_pk = np.ascontiguousarray(whh_pk)
    w_fcT = np.asarray(W_fc, np.float32).T                         # [H, V]
    wfc_pk = np.ascontiguousarray(
        np.concatenate([w_fcT[k * 128:(k + 1) * 128, :] for k in range(HC)],
                       axis=1))
    bih_pk = np.ascontiguousarray(
        np.asarray(b_ih, np.float32).reshape(HC, 128).T)
    bhh_pk = np.ascontiguousarray(
        np.asarray(b_hh, np.float32).reshape(HC, 128).T)
    bfc_rep = np.ascontiguousarray(
        np.broadcast_to(np.asarray(b_fc, np.float32)[None, :], (128, V)))

    in_maps = []
    for c in range(NCORES):
        rows = slice(c * BC, (c + 1) * BC)
        tok_c = tok[rows]                               # [BC, T]
        # tok_pk[p, blk] = tok_c[p % BC, blk*TBLK + p//BC]
        tok_pk = np.ascontiguousarray(
            tok_c.T.reshape(NBLK, TBLK, BC).reshape(NBLK, 128).T)
        hid_c = hid[rows]                               # [BC, H]
        # h0_pk[p, c*BC + b] = hid_c[b, c*128 + p]
        h0_pk = np.ascontiguousarray(
            hid_c.T.reshape(HC, 128, BC).transpose(1, 0, 2).reshape(128, HC * BC))
        in_maps.append({
            "tok_pk": tok_pk, "w_ihT": w_ihT, "whh_pk": whh_pk,
            "wfc_pk": wfc_pk, "bih_pk": bih_pk, "bhh_pk": bhh_pk,
            "bfc_rep": bfc_rep, "h0_pk": h0_pk,
        })
    return in_maps


def kernel(input, hidden, W_ih, W_hh, b_ih, b_hh, W_fc, b_fc):
    from concourse.bass_utils import run_bass_kernel_spmd

    nc = _get_runner()
    in_maps = _pack_inputs(input, hidden, W_ih, W_hh, b_ih, b_hh, W_fc, b_fc)
    res = run_bass_kernel_spmd(nc, in_maps, list(range(NCORES)))
    logits = np.concatenate(
        [res.results[c]["logits"].reshape(BC, T, V) for c in range(NCORES)],
        axis=0)
    h_last = np.concatenate(
        [res.results[c]["h_out"] for c in range(NCORES)], axis=0)[None]
    return logits, h_last


# revision 5
# speedup vs baseline: 11176.9272x; 11176.9272x over previous
"""CharRNN Trainium2 kernel.

Reference computation (per batch row b, t = 0..T-1):
    x_t   = W_ih.T[token[b,t]] + b_ih + b_hh          # row gather  [H]
    h_t   = tanh(x_t + W_hh @ h_{t-1})                # recurrence  [H]
    logit = h_t @ W_fc.T + b_fc                       # output head [V]

Shapes: B=64, T=512, H=512, V=4096.  8 NeuronCores, data-parallel over B
(8 rows per core), zero communication.

Per-core device program (one TileContext, fully unrolled):
  - embedding gather via indirect DMA (128 rows/block, 32 blocks), PE
    transpose into H-partition layout, bias folded into the PSUM-evacuate
    DVE add; x stored fp32 packed per step [j | chunk | b].
  - recurrence keeps h transposed: hT [H on partitions, B free].  Per step:
    16 bf16 matmuls accumulate W_hh.T chunks (first one start=True), one
    fp32 identity-matmul injects x last, one ACT tanh writes hT to SBUF
    (bf16) into per-tranche history tiles.
  - output head: hsT tranche tiles are the stationary lhsT (128-row
    blocks), W_fc.T streams as bf16 moving operand, bias added by the
    mandatory PSUM->SBUF DVE evacuation, 2MB contiguous DMA per row block.
    Row-block groups are interleaved into the next tranche's step loop so
    PE fills recurrence sync gaps with head matmuls.
"""

import threading

import numpy as np

B, T, H, V = 64, 512, 512, 4096
NCORES = 8
BC = B // NCORES          # 8 batch rows per core
HC = H // 128             # 4 hidden chunks
VC = V // 512             # 8 vocab chunks of 512
TBLK = 16                 # gather block = 16 steps * 8 rows = 128 gathered rows
NBLK = T // TBLK          # 32 gather blocks
TRANCHE = 128             # logits row-block granularity (steps per hsT tile)
NTR = T // TRANCHE        # 4 tranches


def _build_nc(reps=1):
    import concourse.bacc as bacc
    import concourse.mybir as mybir
    import concourse.tile as tile
    from concourse import bass
    from concourse.masks import make_identity
    from contextlib import ExitStack

    f32 = mybir.dt.float32
    bf16 = mybir.dt.bfloat16
    i32 = mybir.dt.int32

    nc = bacc.Bacc("TRN2", target_bir_lowering=False, debug=False,
                   num_devices=NCORES)

    # ---- external I/O (per-core) ----
    tok_pk = nc.declare_dram_parameter("tok_pk", [128, NBLK], i32, isOutput=False)
    w_ihT = nc.declare_dram_parameter("w_ihT", [V, H], f32, isOutput=False)
    whh_pk = nc.declare_dram_parameter("whh_pk", [128, 16 * 128], f32, isOutput=False)
    wfc_pk = nc.declare_dram_parameter("wfc_pk", [128, HC * V], f32, isOutput=False)
    bih_pk = nc.declare_dram_parameter("bih_pk", [128, HC], f32, isOutput=False)
    bhh_pk = nc.declare_dram_parameter("bhh_pk", [128, HC], f32, isOutput=False)
    bfc_rep = nc.declare_dram_parameter("bfc_rep", [128, V], f32, isOutput=False)
    h0_pk = nc.declare_dram_parameter("h0_pk", [128, HC * BC], f32, isOutput=False)
    logits = nc.declare_dram_parameter("logits", [BC * T, V], f32, isOutput=True)
    h_out = nc.declare_dram_parameter("h_out", [BC, H], f32, isOutput=True)

    with ExitStack() as ctx:
        tc = ctx.enter_context(tile.TileContext(nc))
        const = ctx.enter_context(tc.tile_pool(name="const", bufs=1))
        gpool = ctx.enter_context(tc.tile_pool(name="gather", bufs=3))
        xpool = ctx.enter_context(tc.tile_pool(name="xp", bufs=3))
        opool = ctx.enter_context(tc.tile_pool(name="outst", bufs=2))
        tpool = ctx.enter_context(tc.tile_pool(name="xt_ps", bufs=2, space="PSUM"))
        zpool = ctx.enter_context(tc.tile_pool(name="z_ps", bufs=3, space="PSUM"))
        lpool = ctx.enter_context(tc.tile_pool(name="l_ps", bufs=3, space="PSUM"))

        for _rep in range(reps):
            emit_one(nc, tc, bass, mybir, make_identity, f32, bf16, i32,
                     const, gpool, xpool, opool, tpool, zpool, lpool,
                     tok_pk, w_ihT, whh_pk, wfc_pk, bih_pk, bhh_pk, bfc_rep,
                     h0_pk, logits, h_out)

    nc.compile()
    return nc


def emit_one(nc, tc, bass, mybir, make_identity, f32, bf16, i32,
             const, gpool, xpool, opool, tpool, zpool, lpool,
             tok_pk, w_ihT, whh_pk, wfc_pk, bih_pk, bhh_pk, bfc_rep,
             h0_pk, logits, h_out):
    # ---- prep ----
    whh_bf = const.tile([128, 16 * 128], bf16, tag="whh", name="whh_bf")
    nc.gpsimd.dma_start(out=whh_bf[:], in_=whh_pk[:])
    h0_bf = const.tile([128, HC * BC], bf16, tag="h0", name="h0_bf")
    nc.gpsimd.dma_start(out=h0_bf[:], in_=h0_pk[:])

    bias_a = const.tile([128, HC], f32, tag="bias_a", name="bias_a")
    bias_b = const.tile([128, HC], f32, tag="bias_b", name="bias_b")
    nc.sync.dma_start(out=bias_a[:], in_=bih_pk[:])
    nc.sync.dma_start(out=bias_b[:], in_=bhh_pk[:])
    bias_pk = const.tile([128, HC], f32, tag="bias", name="bias_pk")
    nc.vector.tensor_add(out=bias_pk[:], in0=bias_a[:], in1=bias_b[:])

    bfc_sb = const.tile([128, V], f32, tag="bfc", name="bfc_sb")
    nc.sync.dma_start(out=bfc_sb[:], in_=bfc_rep[:])

    tok_sb = const.tile([128, NBLK], i32, tag="tok", name="tok_sb")
    nc.sync.dma_start(out=tok_sb[:], in_=tok_pk[:])

    ident_f = const.tile([128, 128], f32, tag="id_f", name="ident_f")
    make_identity(nc, ident_f[:])

    # hsT tranches: [128, HC*BC*TRANCHE] bf16; column = c*1024 + b*128 + (t%128)
    hs = [const.tile([128, HC * BC * TRANCHE], bf16, tag=f"hs{i}",
                     name=f"hs{i}") for i in range(NTR)]
    hs_v = [h[:].rearrange("p (c b t) -> p c b t", c=HC, b=BC, t=TRANCHE)
            for h in hs]

    xp_tiles = {}  # blk -> packed x tile [128, TBLK * HC * BC] f32

    def emit_gather(blk):
        xg = gpool.tile([128, H], f32, tag="xg", name="xg")
        nc.gpsimd.indirect_dma_start(
            out=xg[:], out_offset=None, in_=w_ihT[:],
            in_offset=bass.IndirectOffsetOnAxis(
                ap=tok_sb[:, blk:blk + 1], axis=0))
        # packed x: column = j*(HC*BC) + c*BC + b
        xp = xpool.tile([128, TBLK * HC * BC], f32, tag="xp", name="xp")
        xp_v = xp[:].rearrange("p (j c b) -> p j c b", j=TBLK, c=HC, b=BC)
        for c in range(HC):
            ps = tpool.tile([128, 128], f32, tag="xt", name="xt")
            nc.tensor.transpose(out=ps[:], in_=xg[:, c * 128:(c + 1) * 128],
                                identity=ident_f[:])
            nc.vector.tensor_add(
                out=xp_v[:, :, c, :],
                in0=ps[:].rearrange("p (j b) -> p j b", j=TBLK),
                in1=bias_pk[:, c:c + 1].to_broadcast([128, TBLK, BC]))
        xp_tiles[blk] = xp

    def h_prev(k, t):
        if t == 0:
            return h0_bf[:, k * BC:(k + 1) * BC]
        tr, tt = (t - 1) // TRANCHE, (t - 1) % TRANCHE
        return hs_v[tr][:, k, :, tt]

    def emit_step(t):
        blk, j = t // TBLK, t % TBLK
        z = zpool.tile([128, HC * BC], f32, tag="z", name="z")
        for k in range(HC):
            for m in range(HC):
                nc.tensor.matmul(
                    out=z[:, m * BC:(m + 1) * BC],
                    lhsT=whh_bf[:, (k * HC + m) * 128:(k * HC + m + 1) * 128],
                    rhs=h_prev(k, t),
                    start=(k == 0 and m == 0), stop=False)
        # inject x last: accumulates onto Wh in the same bank
        xp = xp_tiles[blk]
        nc.tensor.matmul(
            out=z[:], lhsT=ident_f[:],
            rhs=xp[:, j * HC * BC:(j + 1) * HC * BC],
            start=False, stop=True)
        tr, tt = t // TRANCHE, t % TRANCHE
        nc.scalar.activation(
            out=hs_v[tr][:, :, :, tt],
            in_=z[:].rearrange("p (c b) -> p c b", c=HC),
            func=mybir.ActivationFunctionType.Tanh)
        if t == T - 1:
            hl = const.tile([128, HC * BC], f32, tag="hlast", name="hlast")
            nc.scalar.activation(out=hl[:], in_=z[:],
                                 func=mybir.ActivationFunctionType.Tanh)
            for c in range(HC):
                nc.sync.dma_start(
                    out=h_out[:, c * 128:(c + 1) * 128].rearrange("b p -> p b"),
                    in_=hl[:, c * BC:(c + 1) * BC])

    out_stages = {}  # rb index -> staging tile

    def emit_logits_group(tr, b, n):
        """One v-chunk group of row block (tr, b): 4 matmuls + DVE evacuate."""
        rb = tr * BC + b
        if n == 0:
            out_stages[rb] = opool.tile([128, V], f32, tag="outst",
                                        name="outst")
        outst = out_stages[rb]
        ps = lpool.tile([128, 512], f32, tag="lg", name="lg")
        for k in range(HC):
            nc.tensor.matmul(
                out=ps[:], lhsT=hs_v[tr][:, k, b, :],
                rhs=wfc_bf[:, k * V + n * 512: k * V + (n + 1) * 512],
                start=(k == 0), stop=(k == HC - 1))
        nc.vector.tensor_add(
            out=outst[:, n * 512:(n + 1) * 512], in0=ps[:],
            in1=bfc_sb[:, n * 512:(n + 1) * 512])
        if n == VC - 1:
            row0 = b * T + tr * TRANCHE
            nc.sync.dma_start(out=logits[row0:row0 + TRANCHE, :], in_=outst[:])
            del out_stages[rb]

    # ---- main schedule ----
    emit_gather(0)
    emit_gather(1)
    # Wfc load is only needed from step ~127 on; emit after first gathers.
    wfc_bf = const.tile([128, HC * V], bf16, tag="wfc", name="wfc_bf")
    nc.gpsimd.dma_start(out=wfc_bf[:], in_=wfc_pk[:])

    # logits groups of tranche tr are spread over the steps of tranche tr+1
    pending = []  # list of (tr, b, n)
    for t in range(T):
        if t % TBLK == 0 and t // TBLK + 2 < NBLK:
            emit_gather(t // TBLK + 2)
        emit_step(t)
        if t % TRANCHE == TRANCHE - 1:
            tr = t // TRANCHE
            pending.extend((tr, b, n) for b in range(BC) for n in range(VC))
        # drain pending at a rate that spreads a tranche (64 groups) over
        # the next 128 steps: one group every other step.
        if pending and t % 2 == 1:
            emit_logits_group(*pending.pop(0))
    while pending:
        emit_logits_group(*pending.pop(0))


_cache = threading.local()


def _get_runner(reps=1):
    cache = getattr(_cache, "runners", None)
    if cache is None:
        cache = _cache.runners = {}
    if reps not in cache:
        cache[reps] = _build_nc(reps)
    return cache[reps]


def _pack_inputs(input, hidden, W_ih, W_hh, b_ih, b_hh, W_fc, b_fc):
    """Host-side layout-only packing (slicing / transpose / replication)."""
    tok = np.asarray(input).astype(np.int32)           # [B, T]
    hid = np.ascontiguousarray(np.asarray(hidden, np.float32)[0])  # [B, H]
    w_ihT = np.ascontiguousarray(np.asarray(W_ih, np.float32).T)   # [V, H]
    w_hhT = np.asarray(W_hh, np.float32).T                         # [H, H]
    # whh_pk column block (k*HC+m) = W_hh.T[k*128:(k+1)*128, m*128:(m+1)*128]
    whh_pk = np.ascontiguousarray(np.concatenate(
        [w_hhT[k * 128:(k + 1) * 128, m * 128:(m + 1) * 128]
         for k in range(HC) for m in range(HC)], axis=1))
    w_fcT = np.asarray(W_fc, np.float32).T                         # [H, V]
    wfc_pk = np.ascontiguousarray(
        np.concatenate([w_fcT[k * 128:(k + 1) * 128, :] for k in range(HC)],
                       axis=1))
    bih_pk = np.ascontiguousarray(
        np.asarray(b_ih, np.float32).reshape(HC, 128).T)
    bhh_pk = np.ascontiguousarray(
        np.asarray(b_hh, np.float32).reshape(HC, 128).T)
    bfc_rep = np.ascontiguousarray(
        np.broadcast_to(np.asarray(b_fc, np.float32)[None, :], (128, V)))

    in_maps = []
    for c in range(NCORES):
        rows = slice(c * BC, (c + 1) * BC)
        tok_c = tok[rows]                               # [BC, T]
        # tok_pk[p, blk] = tok_c[p % BC, blk*TBLK + p//BC]
        tok_pk = np.ascontiguousarray(
            tok_c.T.reshape(NBLK, TBLK, BC).reshape(NBLK, 128).T)
        hid_c = hid[rows]                               # [BC, H]
        # h0_pk[p, c*BC + b] = hid_c[b, c*128 + p]
        h0_pk = np.ascontiguousarray(
            hid_c.T.reshape(HC, 128, BC).transpose(1, 0, 2).reshape(128, HC * BC))
        in_maps.append({
            "tok_pk": tok_pk, "w_ihT": w_ihT, "whh_pk": whh_pk,
            "wfc_pk": wfc_pk, "bih_pk": bih_pk, "bhh_pk": bhh_pk,
            "bfc_rep": bfc_rep, "h0_pk": h0_pk,
        })
    return in_maps


def kernel(input, hidden, W_ih, W_hh, b_ih, b_hh, W_fc, b_fc):
    from concourse.bass_utils import run_bass_kernel_spmd

    nc = _get_runner(1)
    in_maps = _pack_inputs(input, hidden, W_ih, W_hh, b_ih, b_hh, W_fc, b_fc)
    res = run_bass_kernel_spmd(nc, in_maps, list(range(NCORES)))
    logits = np.concatenate(
        [res.results[c]["logits"].reshape(BC, T, V) for c in range(NCORES)],
        axis=0)
    h_last = np.concatenate(
        [res.results[c]["h_out"] for c in range(NCORES)], axis=0)[None]
    return logits, h_last
